# revision 4
# baseline (speedup 1.0000x reference)
"""TRN2 Bass kernel for nn_AttentionModule (dense transformer attention block).

Reference computation (per sample b, x flattened to [256, 4096]):
    proj = conv_w @ x + conv_b                 [32, 4096]
    q    = (q_w @ proj + q_b).T                [4096, 32]
    k    = k_w @ proj + k_b                    [32, 4096]
    v    = v_w @ proj + v_b                    [256, 4096]
    attn = softmax(q @ k, axis=-1)             [4096(n), 4096(m)]
    out  = gamma * (v @ attn.T) + x            [256, 4096]

Sharding: 8 cores = 4 samples x 2 query-halves (2048 queries each). Each core
redundantly computes proj/k/v for its sample (cheap) and its half of the
queries. No cross-core communication. SPMD: odd cores receive x with the
spatial axis rolled by -2048 so "their" queries sit at columns 0:2048;
attention is permutation-invariant over keys so k/v column order is free.

On-core layout: scores are computed transposed, [m_keys(part), n_queries
(free)], so the exp'd scores chunks are directly usable as matmul weights
(lhsT) for the attn@V contraction over m, and the softmax denominator falls
out of the same matmul via an appended ones-column in the V^T projection
(column 256 of the [33,257] rhs; proj carries a ones-row 32 that also folds
in the v bias). No max-subtraction: exp'd scores are stored in bf16 (no
overflow below e^88); numerator and denominator share the same bf16 rounding
so softmax normalization cancels most of it.

The exp stream is split across TWO engines per score group: ACT does a true
exp on the first 1024 columns (m-chunks 4g,4g+1), the DVE does a Schraudolph
bit-trick exp on the last 1024 (m-chunks 4g+2,4g+3): bits = round(s*128/ln2
+ B) as uint16, bitcast to bf16 == 2^(s/ln2) with ~3% max rel error. Softmax
normalization cancels the common-mode part; the end-to-end output error from
this is ~1e-3 (gamma ~0.1 further damps it). This halves the softmax-exp
wall (the single-ACT version serialized ~55us of exp behind the PE).

PSUM budget (8 banks): one 4-bank slot for score-group / prologue 2048-wide
tiles (tag "ps"), two 2-bank slots (tag "po") for attnout accumulators and
all 1024-wide prologue tiles. Per attnout block the emission order is
[SG(2nb) | att 0:16 | SG(2nb+1) | att 16:32 | epilogue] so every PSUM drain
(1.2us exp / 0.7us epilogue) is covered by attnout streaming.

Residual xT and output travel as fp16 with [128, 16, 256] DRAM layout (8KB /
2KB contiguous lines); the host does the transposes. gamma is folded into
v_w/v_b host-side. fp16 feeds the q/k score path.
"""

import numpy as np
from contextlib import ExitStack

import concourse.bass as bass
import concourse.bacc as bacc
import concourse.tile as tile
from concourse import mybir
from concourse.bass_utils import run_bass_kernel_spmd

F32 = mybir.dt.float32
F16 = mybir.dt.float16
BF16 = mybir.dt.bfloat16
U16 = mybir.dt.uint16

B, C, H, W = 4, 256, 64, 64
HW = H * W          # 4096 keys (m)
NQ = HW // 2        # 2048 queries per core (n)
C8 = 32             # qk head dim (e) / proj channels (d)
NSUP = 512          # queries per attention super-block
NBLK = 128          # queries per attnout block
MCH = 128           # keys per m-chunk (one lhsT tile)
N_MCH = HW // MCH   # 32 m-chunks
VN = C + 1          # 257: v channels + ones column (softmax denominator)
NBT = NQ // NBLK    # 16 attnout blocks total

# Schraudolph exp-in-bf16-bits: bits = round(s * 128/ln2 + SCH_B); bitcast
# bf16 ~= e^s (max rel err ~3%). Valid (bits in (0, 32768)) for |s| < 88.
SCH_A = 184.66509904026207
SCH_B = 16250.49

_CACHED = {}


def build_nc():
    nc = bacc.Bacc("TRN2", target_bir_lowering=False, debug=False)
    d_x16 = nc.dram_tensor("x16", [C, HW], F16, kind="ExternalInput").ap()
    d_xT = nc.dram_tensor("xT", [128, NBT, C], F16, kind="ExternalInput").ap()
    d_cwT = nc.dram_tensor("cwT", [2, 128, C8], F16, kind="ExternalInput").ap()
    d_cb = nc.dram_tensor("cb", [C8, 1], F32, kind="ExternalInput").ap()
    # k/q weights carry their bias as row 32, contracted against proj's
    # ones-row — no separate bias op needed.
    d_kwT = nc.dram_tensor("kwT", [C8 + 1, C8], F16, kind="ExternalInput").ap()
    d_qwT = nc.dram_tensor("qwT", [C8 + 1, C8], F16, kind="ExternalInput").ap()
    d_vwb = nc.dram_tensor("vwb", [C8 + 1, VN], F16, kind="ExternalInput").ap()
    d_outT = nc.dram_tensor("outT", [128, NBT, C], F16, kind="ExternalOutput").ap()

    IDENT = mybir.ActivationFunctionType.Identity
    EXP = mybir.ActivationFunctionType.Exp
    MUL = mybir.AluOpType.mult
    ADD = mybir.AluOpType.add

    with tile.TileContext(nc) as tc, ExitStack() as ctx:
        const_pool = ctx.enter_context(tc.tile_pool(name="const", bufs=1))
        big_pool = ctx.enter_context(tc.tile_pool(name="big", bufs=1))

        # ---- constants / inputs ----
        cwT = const_pool.tile([128, 2, C8], F16)
        kwT = const_pool.tile([C8 + 1, C8], F16)
        qwT = const_pool.tile([C8 + 1, C8], F16)
        vwb = const_pool.tile([C8 + 1, VN], F16)
        cb = const_pool.tile([C8, 1], F32)
        warm = const_pool.tile([128, 512], F16)
        for a in range(2):
            nc.sync.dma_start(cwT[:, a, :], d_cwT[a])
        nc.sync.dma_start(kwT[:], d_kwT)
        nc.sync.dma_start(qwT[:], d_qwT)
        nc.sync.dma_start(vwb[:], d_vwb)
        nc.sync.dma_start(cb[:], d_cb)
        nc.gpsimd.memset(warm[:], 0.0)

        # x16: two c-halves [128, HW] fp16 (matmul operand); 1024-col chunks
        # (2KB contiguous lines) across two HWDGE queues; the first proj
        # slice needs only cols 0:2048.
        x16 = [big_pool.tile([128, HW], F16, tag=f"x16_{i}", name=f"x16_{i}")
               for i in range(2)]
        d_x16v = d_x16.rearrange("(a p) m -> a p m", p=128)
        for j in range(4):
            sl = bass.ts(j, HW // 4)
            nc.sync.dma_start(x16[0][:, sl], d_x16v[0][:, sl])
            nc.scalar.dma_start(x16[1][:, sl], d_x16v[1][:, sl])

        # xT: residual input, [128, nb, 256] fp16, one DMA with 8KB lines.
        # On the gpsimd (SWDGE) queue: not needed until the first epilogue.
        xT = big_pool.tile([128, NBT, C], F16)
        nc.gpsimd.dma_start(xT[:], d_xT)

        proj = big_pool.tile([C8 + 1, HW], F16)   # row 32 = ones
        nc.gpsimd.memset(proj[C8 : C8 + 1, :], 1.0)
        k4 = big_pool.tile([128, HW], F16)        # k replicated on 4 row-groups
        qT4 = big_pool.tile([128, NQ], F16)       # query half, replicated x4
        vt = big_pool.tile([128, N_MCH * VN], BF16)  # vT' chunks [m=128, 257]

        # ---- PSUM: one 4-bank score-group slot + two 2-bank slots ----
        psum = ctx.enter_context(tc.tile_pool(name="psum", bufs=1,
                                              space="PSUM"))
        att_pool = ctx.enter_context(tc.tile_pool(name="att", bufs=2))
        out_pool = ctx.enter_context(tc.tile_pool(name="outp", bufs=3))

        def ps_tile(shape, name):
            return psum.tile(shape, F32, tag="ps", bufs=1, name=name)

        def po_tile(shape, name):
            return psum.tile(shape, F32, tag="po", bufs=2, name=name)

        # PE warmup: dummy matmuls on zeros while the input DMAs land, so
        # the HAM clock-gate is released before the real work starts.
        pw = po_tile([C8, 512], "pw")
        for _ in range(8):
            nc.tensor.matmul(pw[:], cwT[:, 0, :], warm[:])

        # proj = conv_w @ x + conv_b (K=256 over 2 chunks); bias applied by
        # ACT on the low 1024 columns, DVE on the high 1024 of each slice.
        def emit_proj_slice(s):
            for h in range(2):
                pp = po_tile([C8, 1024], f"pp{s}{h}")
                for jj in range(2):
                    sl = bass.ts(jj, 512)
                    gsl = bass.ds(s * 2048 + h * 1024 + jj * 512, 512)
                    nc.tensor.matmul(pp[:, sl], cwT[:, 0, :], x16[0][:, gsl],
                                     start=True, stop=False)
                    nc.tensor.matmul(pp[:, sl], cwT[:, 1, :], x16[1][:, gsl],
                                     start=False, stop=True)
                dst = proj[0:C8, bass.ds(s * 2048 + h * 1024, 1024)]
                if h == 0:
                    nc.scalar.activation(dst, pp[:], IDENT, bias=cb[:])
                else:
                    nc.vector.tensor_scalar(dst, pp[:], cb[:], None, ADD)

        # qT4 = q_w' @ proj' (bias via proj ones-row), x4 col-groups
        def emit_q():
            for h in range(2):
                pq = po_tile([128, 1024], f"pq{h}")
                for jj in range(2):
                    sl = bass.ts(jj, 512)
                    psl = bass.ds(h * 1024 + jj * 512, 512)
                    for g in range(4):
                        nc.tensor.matmul(pq[bass.ts(g, 32), sl], qwT[:],
                                         proj[:, psl], tile_position=(0, 32 * g))
                dst = qT4[:, bass.ds(h * 1024, 1024)]
                if h == 0:
                    nc.scalar.copy(dst, pq[:])
                else:
                    nc.vector.tensor_copy(dst, pq[:])

        # k4 = k_w' @ proj' on all 4 col-groups (x4 replication)
        def emit_k_slice(s):
            pk = ps_tile([128, 2048], f"pk{s}")
            for jj in range(4):
                sl = bass.ts(jj, 512)
                gsl = bass.ds(s * 2048 + jj * 512, 512)
                for g in range(4):
                    nc.tensor.matmul(pk[bass.ts(g, 32), sl], kwT[:],
                                     proj[:, gsl], tile_position=(0, 32 * g))
            nc.scalar.copy(k4[:, bass.ds(s * 2048, 1024)], pk[:, 0:1024])
            nc.vector.tensor_copy(k4[:, bass.ds(s * 2048 + 1024, 1024)],
                                  pk[:, 1024:2048])

        # ---- attention ----
        n_sup = NQ // NSUP                # 4 super-blocks of 512 queries
        n_blk = NSUP // NBLK              # 4 attnout blocks per super
        GCH = 4                           # m-chunks per scores group
        n_grp = N_MCH // GCH              # 8 scores groups per super
        e_sbs = {}

        def alloc_e(ns):
            e_sbs[ns] = att_pool.tile([128, N_MCH * NSUP], BF16, tag="e_sb",
                                      name=f"e_sb_{ns}")

        def emit_score_group(ns, g):
            nsl = bass.ts(ns, NSUP)
            e_sb = e_sbs[ns]
            ps = ps_tile([128, GCH * NSUP], f"ps_{ns}_{g}")
            for i in range(GCH):
                mi = GCH * g + i
                nc.tensor.matmul(
                    ps[:, bass.ts(i, NSUP)],
                    k4[bass.ts(i, 32), bass.ts(mi, MCH)],
                    qT4[bass.ts(i, 32), nsl],
                    tile_position=(32 * i, 0),
                )
            base = GCH * g * NSUP
            nc.scalar.activation(e_sb[:, bass.ds(base, 1024)],
                                 ps[:, 0:1024], EXP)
            nc.vector.tensor_scalar(
                e_sb[:, bass.ds(base + 1024, 1024)].bitcast(U16),
                ps[:, 1024:2048], SCH_A, SCH_B, MUL, ADD)

        # scores + exp for super 0 run interleaved with the vT' build; vt
        # copies alternate between ACT and DVE so both engine streams stay
        # balanced with the split exp.
        def emit_v_pair(vg):        # vg in 0..15, chunks 2vg, 2vg+1
            pv = po_tile([128, 2, 512], f"pv{vg}")
            for i in range(2):
                mi = 2 * vg + i
                nc.tensor.matmul(pv[:, i, 0:VN], proj[:, bass.ts(mi, MCH)],
                                 vwb[:])
            vt_sl = vt[:, bass.ds(2 * vg * VN, 2 * VN)].rearrange(
                "p (a v) -> p a v", v=VN)
            if vg % 2 == 0:
                nc.scalar.copy(vt_sl, pv[:, :, 0:VN])
            else:
                nc.vector.tensor_copy(vt_sl, pv[:, :, 0:VN])

        osb_cur = [None]

        def emit_block_epilogue(po, nbg):
            rcol = out_pool.tile([128, 1], F32, tag="rcol",
                                 name=f"rcol_{nbg}")
            nc.vector.reciprocal(rcol[:], po[:, C : C + 1])
            anorm = out_pool.tile([128, C], F32, tag="anorm",
                                  name=f"anorm_{nbg}")
            nc.vector.tensor_scalar_mul(anorm[:], po[:, 0:C], rcol[:])
            if nbg % 2 == 0:
                osb_cur[0] = out_pool.tile([128, 2, C], F16, tag="osb",
                                           name=f"osb_{nbg}")
            osb = osb_cur[0]
            nc.vector.tensor_add(osb[:, nbg % 2, :], anorm[:], xT[:, nbg, :])
            if nbg % 2 == 1:
                nc.sync.dma_start(d_outT[:, nbg - 1 : nbg + 1, :], osb[:])

        # ---- prologue ----
        emit_proj_slice(0)
        emit_q()
        emit_k_slice(0)
        emit_proj_slice(1)
        emit_k_slice(1)
        alloc_e(0)
        for g in range(n_grp):
            emit_score_group(0, g)
            emit_v_pair(2 * g)
            emit_v_pair(2 * g + 1)

        # ---- steady state ----
        # Per block: [SG(ns+1, 2nb) | att 0:16 | SG(ns+1, 2nb+1) | att 16:32
        # | epilogue]; every PSUM drain is covered by attnout streaming.
        def emit_att_half(po, e_sb, nb, lo):
            for mi in range(lo, lo + N_MCH // 2):
                nc.tensor.matmul(
                    po[:],
                    e_sb[:, bass.ds(mi * NSUP + nb * NBLK, NBLK)],
                    vt[:, bass.ts(mi, VN)],
                    start=(mi == 0), stop=(mi == N_MCH - 1),
                )

        for ns in range(n_sup):
            if ns + 1 < n_sup:
                alloc_e(ns + 1)
            for nb in range(n_blk):
                e_sb = e_sbs[ns]
                po = po_tile([128, VN], f"po_{ns}_{nb}")
                if ns + 1 < n_sup:
                    emit_score_group(ns + 1, 2 * nb)
                emit_att_half(po, e_sb, nb, 0)
                if ns + 1 < n_sup:
                    emit_score_group(ns + 1, 2 * nb + 1)
                emit_att_half(po, e_sb, nb, N_MCH // 2)
                emit_block_epilogue(po, ns * n_blk + nb)
            e_sbs.pop(ns)

    nc.compile()
    return nc


def _prep_in_maps(x, conv_w, conv_b, q_w, q_b, k_w, k_b, v_w, v_b, gamma):
    g = np.float32(gamma[0])
    cwT = np.ascontiguousarray(conv_w.T.reshape(2, 128, C8)).astype(np.float16)
    kwT = np.concatenate([k_w.T, k_b[None, :]], axis=0).astype(np.float16)
    qwT = np.concatenate([q_w.T, q_b[None, :]], axis=0).astype(np.float16)
    vwb = np.zeros((C8 + 1, VN), np.float16)
    vwb[0:C8, 0:C] = (g * v_w).T.astype(np.float16)
    vwb[C8, 0:C] = (g * v_b).astype(np.float16)
    vwb[C8, C] = 1.0
    cb = conv_b.reshape(C8, 1).astype(np.float32)

    in_maps = []
    for core in range(8):
        b, hf = core // 2, core % 2
        xf = np.asarray(x[b], np.float32).reshape(C, HW)
        if hf:
            # rotate spatial columns: this core's query half -> cols 0:2048
            xf = np.roll(xf, -NQ, axis=1)
        xTh = np.ascontiguousarray(
            xf[:, 0:NQ].T.reshape(NBT, 128, C).transpose(1, 0, 2)
        ).astype(np.float16)
        in_maps.append({
            "x16": np.ascontiguousarray(xf).astype(np.float16),
            "xT": xTh,
            "cwT": cwT, "cb": cb, "kwT": kwT, "qwT": qwT, "vwb": vwb,
        })
    return in_maps


def kernel(x, conv_w, conv_b, q_w, q_b, k_w, k_b, v_w, v_b, gamma, **run_kw):
    if "nc" not in _CACHED:
        _CACHED["nc"] = build_nc()
    nc = _CACHED["nc"]
    in_maps = _prep_in_maps(x, conv_w, conv_b, q_w, q_b, k_w, k_b, v_w, v_b,
                            gamma)
    res = run_bass_kernel_spmd(nc, in_maps, core_ids=list(range(8)), **run_kw)
    _CACHED["last_result"] = res
    out = np.empty((B, C, HW), np.float32)
    for core in range(8):
        b, hf = core // 2, core % 2
        oc = np.asarray(res.results[core]["outT"])      # [128, 16, 256] fp16
        ocf = oc.astype(np.float32).transpose(1, 0, 2).reshape(NQ, C)
        out[b, :, hf * NQ : (hf + 1) * NQ] = ocf.T
    return out.reshape(B, C, H, W)
